# revision 1
# baseline (speedup 1.0000x reference)
"""CRF loss (forward-algorithm partition function minus gold score, batch mean)
on 8 Trainium2 NeuronCores.

Strategy: pure data parallel over batch (512 -> 64 per core).

Per-core math (exp-space reformulation of the log-space recurrence):
    fv_{s+1}[n] = feat_s[n] + LSE_p(trans[n,p] + fv_s[p])
becomes, with e = exp(fv - running_shift):
    e_{s+1} = exp(feat_s) * (M @ e_s),   M[n,p] = exp(trans[n,p] - c)
One 64x64 matmul + one elementwise multiply per step; a constant log-shift c
per step is folded into M, and an exact column-sum renorm every 128 steps
keeps everything in f32 range (numerically validated: inter-renorm drift
stays within e^-4..e^+10 for c=5.1). The renorm z's are stashed and a single
Ln at the end recovers sum(log z) + log(terminal), avoiding ACT table churn.

Layouts: state is tag-major (prev-tag on partitions, batch on free dim).
Steps ping-pong between partition halves 0-63 / 64-127 so the matmul
(PE quadrant via tile_position) and the DMA-transposed exp(feat) tiles
always line up lane-for-lane. The 64 batches are split into two independent
32-batch chains (A: cols 0-31, B: cols 32-63) with separate state tiles and
PSUM banks so the two serial dependence chains interleave on PE/DVE.

feats stream in bf16 (halves HBM traffic; rel-err ~1e-7 since forward and
gold share the quantization). Transposition uses the DMA XBAR in two big
blocked-transpose instructions per chunk (cost is ~1.8us fixed + 14ns per
16x128 tile, so batching 16 pair-blocks into one instruction is ~16x
cheaper than per-pair transposes).

Gold score: gpsimd indirect_copy gathers. transitions[cur,prev] comes from a
partition-replicated flat table (group-shared indices are then all valid);
feats[b,s,cur] is gathered per 16-partition group with a periodic 0/1 mask
selecting the lane whose batch matches the index.
"""

import numpy as np
import ml_dtypes
from contextlib import ExitStack

import concourse.bass as bass
import concourse.tile as tile
from concourse import bacc, mybir
from concourse.bass_utils import run_bass_kernel_spmd

F32 = mybir.dt.float32
BF16 = mybir.dt.bfloat16
U16 = mybir.dt.uint16

B, S, T = 512, 1024, 64
NCORES = 8
BS = B // NCORES          # 64 batches per core
HB = BS // 2              # 32 batches per chain
START_TAG, STOP_TAG = 62, 63
CSHIFT = 5.1              # per-step constant log shift folded into M
RENORM = 128              # renorm period (steps)
NREN = S // RENORM - 1    # 7 renorms (none before the terminal)
W = 64                    # sequence steps per feats chunk
NCHUNK = S // W           # 16
NPAIR = S + 1             # transition pairs per batch incl. terminal STOP pair
TPG = NPAIR * (BS // 8)   # trans pairs per 16-partition group (8 b's each)
TPAD = -(-TPG // 1024) * 1024  # padded to the 1024-elems-per-IndirectCopy limit


def crf_kernel(ctx: ExitStack, tc: tile.TileContext, outs, ins,
               gold=True, chain=True, tpose=True):
    nc = tc.nc
    (fwd_o, esum_o, tsum_o) = outs
    (featsbf, transT, stopcol, init, transtab_i, emitidx_i, emitmask_i,
     transidx_i) = ins

    const = ctx.enter_context(tc.tile_pool(name="const", bufs=1))
    natp = ctx.enter_context(tc.tile_pool(name="nat", bufs=3))
    tpp = ctx.enter_context(tc.tile_pool(name="tp", bufs=3))
    efp = ctx.enter_context(tc.tile_pool(name="ef", bufs=3))
    idxp = ctx.enter_context(tc.tile_pool(name="idx", bufs=2))
    egp = ctx.enter_context(tc.tile_pool(name="eg", bufs=2))
    qap = ctx.enter_context(tc.tile_pool(name="qa", bufs=2, space="PSUM"))
    qbp = ctx.enter_context(tc.tile_pool(name="qb", bufs=2, space="PSUM"))
    zp = ctx.enter_context(tc.tile_pool(name="z", bufs=2, space="PSUM"))
    rbp = ctx.enter_context(tc.tile_pool(name="rb", bufs=2, space="PSUM"))
    smp = ctx.enter_context(tc.tile_pool(name="sm", bufs=2))

    # ---- constants / one-time setup ----
    mtraw = const.tile([128, T], F32)
    nc.sync.dma_start(mtraw[0:64, :], transT[:, :])
    nc.sync.dma_start(mtraw[64:128, :], transT[:, :])
    negc = const.tile([128, 1], F32)
    nc.vector.memset(negc[:, :], -CSHIFT)
    mt = const.tile([128, T], F32)   # exp(trans.T - c), both halves
    nc.scalar.activation(mt[:, :], mtraw[:, :],
                         mybir.ActivationFunctionType.Exp, bias=negc[:, :])

    stopraw = const.tile([128, 1], F32)
    nc.sync.dma_start(stopraw[64:128, :], stopcol[:, :])
    stopt = const.tile([128, 1], F32)
    nc.scalar.activation(stopt[64:128, :], stopraw[64:128, :],
                         mybir.ActivationFunctionType.Exp)

    ones_col = const.tile([128, 1], F32)
    nc.vector.memset(ones_col[:, :], 1.0)
    ones_row = const.tile([1, T], F32)
    nc.vector.memset(ones_row[:, :], 1.0)

    # two independent 32-batch chains; state_0 lives at half 1
    stA = const.tile([128, HB], F32)
    stB = const.tile([128, HB], F32)
    nc.vector.memset(stA[0:64, :], 0.0)
    nc.vector.memset(stB[0:64, :], 0.0)
    nc.sync.dma_start(stA[64:128, :], init[:, 0:HB])
    nc.sync.dma_start(stB[64:128, :], init[:, HB:BS])
    sts = (stA, stB)

    # log-z stash: NREN renorm slots + 1 terminal slot, [A(32) | B(32)] each
    zbuf = const.tile([1, (NREN + 1) * BS], F32)

    transtab = const.tile([128, 4100], F32)
    nc.sync.dma_start(transtab[:, :], transtab_i[:, :])
    emitmask = const.tile([128, W * 8], F32)
    nc.sync.dma_start(emitmask[:, :], emitmask_i[:, :])
    esums = const.tile([128, NCHUNK], F32)

    # ---- gold transitions term: group-shared gathers + one accum ----
    if not gold:
        nc.vector.memset(esums[:, :], 0.0)
    tsum = const.tile([128, 1], F32)
    if gold:
        tidx = const.tile([128, TPAD // 16], U16)
        nc.sync.dma_start(tidx[:, :], transidx_i[:, :])
        tgath = const.tile([128, TPAD], F32)
        for t in range(TPAD // 1024):
            nc.gpsimd.indirect_copy(tgath[:, 1024 * t:1024 * (t + 1)],
                                    transtab[:, :],
                                    tidx[:, 64 * t:64 * (t + 1)], True)
        nc.scalar.activation(tgath[:, :], tgath[:, :],
                             mybir.ActivationFunctionType.Copy,
                             accum_out=tsum[:, :])
    else:
        nc.vector.memset(tsum[:, :], 0.0)
    nc.sync.dma_start(tsum_o[:, :], tsum[:, :])

    # ---- main streaming loop over 16 chunks of 64 steps ----
    for k in range(NCHUNK):
        # natural-layout bf16 chunk: partitions = (s_half, b), free = 32*64
        nat = natp.tile([128, W * 32], BF16)
        src = featsbf[:, k * W * T:(k + 1) * W * T]
        nc.sync.dma_start(nat[:, :], src.rearrange("b (h f) -> h b f", h=2))

        # gold emit gather for this chunk (off the critical path)
        if gold:
            eidx = idxp.tile([128, W // 2], U16)
            nc.sync.dma_start(eidx[:, :],
                              emitidx_i[:, k * (W // 2):(k + 1) * (W // 2)])
            eg = egp.tile([128, W * 8], BF16)
            nc.gpsimd.indirect_copy(eg[:, :], nat[:, :], eidx[:, :], True)
            egf = egp.tile([128, W * 8], F32, tag="egf")
            nc.scalar.activation(egf[:, :], eg[:, :],
                                 mybir.ActivationFunctionType.Copy)
            egm = egp.tile([128, W * 8], F32, tag="egm")
            nc.gpsimd.tensor_tensor(egm[:, :], egf[:, :], emitmask[:, :],
                                    op=mybir.AluOpType.mult)
            nc.scalar.activation(egm[:, :], egm[:, :],
                                 mybir.ActivationFunctionType.Copy,
                                 accum_out=esums[:, k:k + 1])

        # blocked DMA transposes (one per s-half) + one bulk Exp
        if not tpose:
            continue
        tp = tpp.tile([128, W * T // 2], BF16)
        for h in range(2):
            out3d = tp[:, 16 * h * T:(16 * h + 16) * T].rearrange(
                "p (j t) -> p j t", t=T)
            eng = nc.sync if h == 0 else nc.scalar
            eng.dma_start(out3d, nat[h * 64:(h + 1) * 64, :], transpose=True)
        ef = efp.tile([128, W * T // 2], F32)
        nc.scalar.activation(ef[:, :], tp[:, :],
                             mybir.ActivationFunctionType.Exp)

        # ---- two interleaved serial chains: matmul + multiply per step ----
        if not chain:
            continue
        for sl in range(W):
            s = k * W + sl
            hs = s % 2          # half where q / expfeat / new state live
            hr = 1 - hs         # half where the current state lives
            j = sl // 2
            for x, (st, qp_) in enumerate(((stA, qap), (stB, qbp))):
                q = qp_.tile([128, HB], F32)
                nc.tensor.matmul(q[hs * 64:hs * 64 + 64, :],
                                 mt[hr * 64:hr * 64 + 64, :],
                                 st[hr * 64:hr * 64 + 64, :],
                                 tile_position=(hr * 64, hs * 64))
                nc.vector.tensor_tensor(
                    st[hs * 64:hs * 64 + 64, :],
                    q[hs * 64:hs * 64 + 64, :],
                    ef[hs * 64:hs * 64 + 64, j * T + x * HB:j * T + (x + 1) * HB],
                    op=mybir.AluOpType.mult)
            if (s + 1) % RENORM == 0 and s != S - 1:
                # column-sum renorm; renorm steps are odd -> state at half 1
                r = (s + 1) // RENORM - 1
                for x, st in enumerate(sts):
                    z = zp.tile([1, HB], F32)
                    nc.tensor.matmul(z[:, :], ones_col[64:128, :],
                                     st[64:128, :], tile_position=(64, 0))
                    nc.vector.tensor_copy(
                        zbuf[0:1, r * BS + x * HB:r * BS + (x + 1) * HB],
                        z[:, :])
                    rz = smp.tile([1, HB], F32, tag="rz")
                    nc.vector.reciprocal(rz[:, :], z[:, :])
                    rb = rbp.tile([128, HB], F32)
                    nc.tensor.matmul(rb[64:128, :], ones_row[:, :], rz[:, :],
                                     tile_position=(0, 64))
                    nc.vector.tensor_tensor(st[64:128, :], st[64:128, :],
                                            rb[64:128, :],
                                            op=mybir.AluOpType.mult)

    # ---- terminal: z_term = sum_n exp(trans[STOP,n]) * state[n] ----
    for x, st in enumerate(sts):
        tq = zp.tile([1, HB], F32, tag="z")
        nc.tensor.matmul(tq[:, :], stopt[64:128, :], st[64:128, :],
                         tile_position=(64, 0))
        nc.vector.tensor_copy(
            zbuf[0:1, NREN * BS + x * HB:NREN * BS + (x + 1) * HB], tq[:, :])

    # fwd[b] = sum_r ln(z_r[b])  (renorm z's + terminal z)
    lnz = smp.tile([1, (NREN + 1) * BS], F32, tag="lnz")
    nc.scalar.activation(lnz[:, :], zbuf[:, :],
                         mybir.ActivationFunctionType.Ln)
    fwd = smp.tile([1, BS], F32, tag="fwd")
    lnz_v = lnz[0:1, :].rearrange("p (r c) -> p c r", c=BS)
    nc.vector.tensor_reduce(fwd[:, :], lnz_v, axis=mybir.AxisListType.X,
                            op=mybir.AluOpType.add)
    nc.sync.dma_start(fwd_o[:, :], fwd[:, :])

    esum = const.tile([128, 1], F32)
    nc.vector.tensor_reduce(esum[:, :], esums[:, :],
                            axis=mybir.AxisListType.X, op=mybir.AluOpType.add)
    nc.sync.dma_start(esum_o[:, :], esum[:, :])


def build(gold=True, chain=True, tpose=True):
    nc = bacc.Bacc("TRN2", target_bir_lowering=False, debug=False)
    ins_spec = [
        ("featsbf", [BS, S * T], BF16),
        ("transT", [T, T], F32),
        ("stopcol", [T, 1], F32),
        ("init", [T, BS], F32),
        ("transtab", [128, 4100], F32),
        ("emitidx", [128, NCHUNK * W // 2], U16),
        ("emitmask", [128, W * 8], F32),
        ("transidx", [128, TPAD // 16], U16),
    ]
    outs_spec = [
        ("fwd", [1, BS], F32),
        ("esum", [128, 1], F32),
        ("tsum", [128, 1], F32),
    ]
    ins = [nc.declare_dram_parameter(n, s, d, isOutput=False).ap()
           for n, s, d in ins_spec]
    outs = [nc.declare_dram_parameter(n, s, d, isOutput=True).ap()
            for n, s, d in outs_spec]
    with tile.TileContext(nc) as tc:
        with ExitStack() as ctx:
            crf_kernel(ctx, tc, outs, ins, gold=gold, chain=chain, tpose=tpose)
    nc.compile()
    return nc


def host_prep(feats, transitions, tags, mask):
    """Build the 8 per-core input maps."""
    assert feats.shape == (B, S, T) and transitions.shape == (T, T)
    mask_arr = np.asarray(mask)
    assert np.all(mask_arr == 1), "kernel assumes an all-ones mask"
    feats = np.asarray(feats, dtype=np.float32)
    transitions = np.asarray(transitions, dtype=np.float32)
    tags = np.asarray(tags).astype(np.int64)

    transT = np.ascontiguousarray(transitions.T)
    stopcol = np.ascontiguousarray(transitions[STOP_TAG, :].reshape(T, 1))
    init = np.zeros((T, BS), np.float32)
    init[START_TAG, :] = 1.0
    ttab = np.zeros((128, 4100), np.float32)
    ttab[:, :4096] = transitions.reshape(1, 4096)

    emitmask = np.zeros((128, W * 8), np.float32)
    p_ = np.arange(128)[:, None]
    i_ = np.arange(W * 8)[None, :]
    emitmask[(p_ % 16) == (i_ % 16)] = 1.0

    in_maps = []
    for c in range(NCORES):
        b0 = c * BS
        fb = feats[b0:b0 + BS].reshape(BS, S * T).astype(ml_dtypes.bfloat16)
        tg = tags[b0:b0 + BS]

        # emit gather indices: EIDX[p, k*32 + col] = col*64 + cur[b, s]
        # with b = p%64, h = p//64, s = k*64 + h*32 + col
        eidx = np.zeros((128, NCHUNK * W // 2), np.uint16)
        p_idx = np.arange(128)
        b_of_p = 16 * ((p_idx // 16) % 4) + (p_idx % 16)
        h_of_p = p_idx // 64
        for k in range(NCHUNK):
            for col in range(W // 2):
                s = k * W + h_of_p * 32 + col
                eidx[:, k * (W // 2) + col] = col * T + tg[b_of_p, s]

        # transition-pair gather indices, group-shared (all lanes valid)
        cur = np.concatenate([tg, np.full((BS, 1), STOP_TAG, np.int64)], 1)
        prev = np.concatenate([np.full((BS, 1), START_TAG, np.int64), tg], 1)
        lin = (cur * T + prev).astype(np.uint16)        # (BS, S+1)
        tidx = np.full((128, TPAD // 16), 4096, np.uint16)  # pad -> zero entry
        for g in range(8):
            lst = lin[8 * g:8 * g + 8].reshape(-1)      # 8 b's x 1025, b-major
            n = lst.shape[0]
            ii = np.arange(n)
            tidx[16 * g + ii % 16, ii // 16] = lst
        in_maps.append({
            "featsbf": fb, "transT": transT, "stopcol": stopcol, "init": init,
            "transtab": ttab, "emitidx": eidx, "emitmask": emitmask,
            "transidx": tidx,
        })
    return in_maps


def host_finish(results):
    fwd_total = 0.0
    gold_total = 0.0
    for r in results:
        fwd_total += float(r["fwd"].astype(np.float64).sum()) + BS * S * CSHIFT
        gold_total += float(r["esum"].astype(np.float64).sum())
        gold_total += float(r["tsum"][::16, 0].astype(np.float64).sum())
    return np.asarray((fwd_total - gold_total) / B, dtype=np.float32)


_NC = None


def kernel(feats, transitions, tags, mask):
    global _NC
    if _NC is None:
        _NC = build()
    in_maps = host_prep(feats, transitions, tags, mask)
    res = run_bass_kernel_spmd(_NC, in_maps, list(range(NCORES)))
    return host_finish(res.results)


if __name__ == "__main__":
    import reference
    inp = reference.setup_inputs()
    out = kernel(**{k: np.asarray(v) for k, v in inp.items()})
    print("kernel loss:", out)



# revision 22
# speedup vs baseline: 3.6009x; 3.6009x over previous
"""CRF loss (forward-algorithm partition function minus gold score, batch mean)
on 8 Trainium2 NeuronCores.

Strategy: pure data parallel over batch (512 -> 64 per core), plus a 16-way
SEQUENCE split per core to break the serial recurrence.

Per-core math (exp-space reformulation of the log-space recurrence):
    e_{s+1} = exp(feat_s) * (M @ e_s),   M[n,p] = exp(trans[n,p] - c)
Products of positive matrices contract to rank-1 exponentially fast, so the
1024-step chain is split into 16 segments of 64 steps. Each segment's chain
starts DELTA=8 steps early from a uniform vector ("warmup"): after 8 steps
the state direction matches the true forward message to ~1e-6 (validated in
numpy); only the scale is off. Scales are stitched with column-sum ratios:
    log z = log(stop . u_15) + sum_k [log t_{k-1} - log h_k]
where t_k = colsum of chain k's final state and h_k = colsum of chain k's
state at warmup end. Chain 0 needs no warmup (it is reset to the exact
e_start at round DELTA).

The 16 chains run as 4 independent LANES (2 gangs x {DVE, Pool}): per round
each lane does one bf16 matmul (PE) + one elementwise multiply (its engine).
Lanes have separate state/psum tiles so the Tile framework inserts no
cross-lane hazards; the round-critical path of each lane hides behind the
other lanes' engine work. exp(feat) runs on the Activation engine in-loop
over 4-round blocks, reading a host-pre-transposed bf16 feats buffer
(tag-major), so there are no device-side DMA transposes.

Gold score:
  - emit: per chunk, one gpsimd indirect_copy gathers feats[b,s,tag] from the
    tag-major feats buffer with group-shared, lane-bucketed indices + a
    periodic 0/1 mask, reduced by one DVE tensor_tensor_reduce per chunk.
    Pad slots point at slice col 0 and are subtracted on the host (exact,
    computed from the same bf16 values the device gathers).
  - transitions: sum trans[cur,prev] = <histogram(cur*64+prev), trans> — the
    host counts index pairs (pure index prep), the device does one
    tensor_tensor_reduce dot product against the replicated table.
"""

import numpy as np
import ml_dtypes
from contextlib import ExitStack

import concourse.bass as bass
import concourse.tile as tile
from concourse import bacc, mybir
from concourse.bass_utils import run_bass_kernel_spmd

F32 = mybir.dt.float32
BF16 = mybir.dt.bfloat16
U16 = mybir.dt.uint16

B, S, T = 512, 1024, 64
NCORES = 8
BS = B // NCORES          # 64 batches per core
START_TAG, STOP_TAG = 62, 63
CSHIFT = 5.1              # per-step constant log shift folded into M

K = 16                    # sequence segments (= 64-step chunks)
L = S // K                # 64 steps per segment
DELTA = 8                 # warmup steps per chain (must be even, mult of 4)
R = L + DELTA             # 72 rounds
G = 8                     # chains per gang
NG = K // G               # 2 gangs
GW = G * BS               # 512 cols per gang
CCH = (L // 2) * BS       # 2048 cols per chunk in featT
NCHUNK = K
CH_DVE = 6                # chains per gang multiplied on DVE (rest on Pool)
EMW = 16 * BS             # 1024: emit gather stream width per chunk


def crf_kernel(ctx: ExitStack, tc: tile.TileContext, outs, ins):
    nc = tc.nc
    (fwd_o, esum_o, tsum_o) = outs
    (featT_i, transT_i, stopcol_i, initcol_i, table_i, counts_i,
     eidx_i, emitmask_i) = ins

    const = ctx.enter_context(tc.tile_pool(name="const", bufs=1))
    efp = ctx.enter_context(tc.tile_pool(name="ef", bufs=3))
    egp = ctx.enter_context(tc.tile_pool(name="eg", bufs=2))
    scr = ctx.enter_context(tc.tile_pool(name="scr", bufs=2))
    qp = [ctx.enter_context(tc.tile_pool(name=f"q{g}", bufs=1, space="PSUM"))
          for g in range(NG)]
    qsp = [ctx.enter_context(tc.tile_pool(name=f"qs{g}", bufs=2))
           for g in range(NG)]
    zp = ctx.enter_context(tc.tile_pool(name="z", bufs=1, space="PSUM"))
    smp = ctx.enter_context(tc.tile_pool(name="sm", bufs=2))

    # ---- constants ----
    mtraw = const.tile([128, T], F32)
    nc.sync.dma_start(mtraw[0:64, :], transT_i[:, :])
    nc.sync.dma_start(mtraw[64:128, :], transT_i[:, :])
    negc = const.tile([128, 1], F32)
    nc.vector.memset(negc[:, :], -CSHIFT)
    mt = const.tile([128, T], BF16)   # exp(trans.T - c), both halves, bf16
    nc.scalar.activation(mt[:, :], mtraw[:, :],
                         mybir.ActivationFunctionType.Exp, bias=negc[:, :])

    stopraw = const.tile([128, 1], F32)
    nc.sync.dma_start(stopraw[64:128, :], stopcol_i[:, :])
    stopt = const.tile([128, 1], BF16)
    nc.scalar.activation(stopt[64:128, :], stopraw[64:128, :],
                         mybir.ActivationFunctionType.Exp)

    ones_col = const.tile([128, 1], BF16)
    nc.vector.memset(ones_col[:, :], 1.0)
    initcol = const.tile([128, BS], BF16)  # e_start pattern in rows 64:128
    nc.sync.dma_start(initcol[64:128, :], initcol_i[:, :])

    table = const.tile([128, 256], F32)
    nc.sync.dma_start(table[:, :], table_i[:, :])
    counts = const.tile([128, 256], BF16)
    nc.sync.dma_start(counts[:, :], counts_i[:, :])
    eidx = const.tile([128, NCHUNK * BS], U16)
    nc.sync.dma_start(eidx[:, :], eidx_i[:, :])
    emitmask = const.tile([128, EMW], BF16)
    nc.sync.dma_start(emitmask[:, :], emitmask_i[:, :])

    # gold transitions: one dot product against the replicated table
    tsc = const.tile([128, 256], F32)
    tsum = const.tile([128, 1], F32)
    nc.vector.scalar_tensor_tensor(tsc[:, :], table[:, :], 1.0, counts[:, :],
                                   op0=mybir.AluOpType.mult,
                                   op1=mybir.AluOpType.mult,
                                   accum_out=tsum[:, :])
    nc.sync.dma_start(tsum_o[:, :], tsum[:, :])

    # ---- feats (tag-major bf16, host-transposed), streamed per chunk ----
    featT = const.tile([128, (NCHUNK + 1) * CCH], BF16)
    nc.vector.memset(featT[:, 0:CCH], 0.0)           # warmup chunk: feat = 0
    for c in range(1, NCHUNK + 1):
        nc.sync.dma_start(featT[:, c * CCH:(c + 1) * CCH],
                          featT_i[:, (c - 1) * CCH:c * CCH])
    featT3 = featT[:, :].rearrange("p (c x) -> p c x", x=CCH)

    # ---- 4 lanes: (gang g, engine e). e=0: DVE TT reads PSUM directly,
    #      chains [0, CH_DVE). e=1: Act copies PSUM->SBUF, Pool TT multiplies
    #      (GPSIMD cannot touch PSUM), chains [CH_DVE, G).
    #      Each lane owns its state + psum tiles (no cross-lane hazards).
    lanes = []
    for g in range(NG):
        for e, (i0, i1) in enumerate(((0, CH_DVE), (CH_DVE, G))):
            w = (i1 - i0) * BS
            st = const.tile([128, w], BF16, tag=f"st{g}{e}")
            nc.vector.memset(st[0:64, :], 1.0 / T)
            nc.vector.memset(st[64:128, :], 1.0 / T)
            lanes.append(dict(g=g, e=e, i0=i0, i1=i1, w=w, st=st))

    # log-scale stash: [h(16 chains) | t(16 chains)] x 64 batches
    zbuf = const.tile([1, 2 * K * BS], F32)

    esums = const.tile([128, NCHUNK], F32)

    NBLK = R // 4
    ef_blks = [[None] * NBLK for _ in range(NG)]

    def issue_exp(g, m):
        # ef block (4 rounds): exp of the 8 chains' 2-column (128-elem) window
        eb = efp.tile([128, G * 2 * BS], BF16, tag=f"ef{g}_{m % 3}")
        r0 = 4 * m
        off = 0 if r0 < DELTA else 1
        j0 = (L - DELTA + r0) // 2 if r0 < DELTA else (r0 - DELTA) // 2
        i0 = G * g + off
        src3 = featT3[:, i0:i0 + G, j0 * BS:(j0 + 2) * BS]
        nc.scalar.activation(eb[:, :].rearrange("p (i x) -> p i x", x=2 * BS),
                             src3, mybir.ActivationFunctionType.Exp)
        ef_blks[g][m] = eb

    for g in range(NG):
        issue_exp(g, 0)
        issue_exp(g, 1)

    def issue_emit(c):
        eg = egp.tile([128, EMW], BF16)
        nc.gpsimd.indirect_copy(eg[:, :],
                                featT[:, (c + 1) * CCH:(c + 2) * CCH],
                                eidx[:, c * BS:(c + 1) * BS], True)
        sc = scr.tile([128, EMW], BF16)
        nc.vector.scalar_tensor_tensor(sc[:, :], eg[:, :], 1.0, emitmask[:, :],
                                       op0=mybir.AluOpType.mult,
                                       op1=mybir.AluOpType.mult,
                                       accum_out=esums[:, c:c + 1])

    for c in range(6):
        issue_emit(c)
    emit_next = 6

    def capture(dst_off):
        # column-sums of all 16 chains' states into zbuf[dst_off:dst_off+1024]
        for g in range(NG):
            z = zp.tile([1, GW], F32, tag=f"z{g}")
            for lane in lanes:
                if lane["g"] != g:
                    continue
                nc.tensor.matmul(z[0:1, lane["i0"] * BS:lane["i1"] * BS],
                                 ones_col[64:128, :], lane["st"][64:128, :],
                                 tile_position=(64, 0))
            nc.vector.tensor_copy(
                zbuf[0:1, dst_off + g * GW:dst_off + (g + 1) * GW], z[:, :])

    # ---- main loop ----
    for r in range(R):
        hs = r % 2
        hr = 1 - hs
        m, jj = r // 4, (r % 4) // 2
        if r % 4 == 0 and m + 2 < NBLK:
            for g in range(NG):
                issue_exp(g, m + 2)
        if r % 4 == 2 and emit_next < NCHUNK:
            issue_emit(emit_next)
            emit_next += 1
        if r == DELTA:
            # h-capture at position 64k (state after round DELTA-1, half 1)
            capture(0)
            # chain-0 reset to the exact e_start (gang 0, DVE lane, chain 0)
            nc.scalar.copy(lanes[0]["st"][64:128, 0:BS], initcol[64:128, :])
        for lane in lanes:
            g, i0, i1, w, st = (lane["g"], lane["i0"], lane["i1"],
                                lane["w"], lane["st"])
            q = qp[g].tile([128, w], F32, tag=f"q{lane['e']}")
            nc.tensor.matmul(q[hs * 64:hs * 64 + 64, :],
                             mt[hr * 64:hr * 64 + 64, :],
                             st[hr * 64:hr * 64 + 64, :],
                             tile_position=(hr * 64, hs * 64))
            eb = ef_blks[g][m]
            ebv = eb[hs * 64:hs * 64 + 64, :].rearrange(
                "p (i y b) -> p i y b", i=G, y=2)[:, :, jj, :]
            if lane["e"] == 0:
                src = q
            else:
                src = qsp[g].tile([128, w], BF16, tag=f"qs{g}")
                nc.scalar.copy(src[hs * 64:hs * 64 + 64, :],
                               q[hs * 64:hs * 64 + 64, :])
            eng = nc.vector if lane["e"] == 0 else nc.gpsimd
            eng.tensor_tensor(
                st[hs * 64:hs * 64 + 64, :].rearrange(
                    "p (i b) -> p i b", b=BS),
                src[hs * 64:hs * 64 + 64, :].rearrange(
                    "p (i b) -> p i b", b=BS),
                ebv[:, i0:i1, :],
                op=mybir.AluOpType.mult)

    while emit_next < NCHUNK:
        issue_emit(emit_next)
        emit_next += 1

    # ---- t-capture (final states live in half 1 after round R-1) ----
    capture(K * BS)
    # chain 15 terminal: stop . state overwrites its t slot
    last = lanes[-1]
    tq = zp.tile([1, BS], F32, tag="tq")
    nc.tensor.matmul(tq[:, :], stopt[64:128, :],
                     last["st"][64:128, last["w"] - BS:last["w"]],
                     tile_position=(64, 0))
    nc.vector.tensor_copy(zbuf[0:1, 2 * K * BS - BS:2 * K * BS], tq[:, :])

    # ---- fwd[b] = sum_k ln t'_k[b] - sum_{k>=1} ln h_k[b] ----
    lnz = smp.tile([1, 2 * K * BS], F32, tag="lnz")
    nc.scalar.activation(lnz[:, :], zbuf[:, :],
                         mybir.ActivationFunctionType.Ln)
    red_t = smp.tile([1, BS], F32, tag="rt")
    nc.vector.tensor_reduce(
        red_t[:, :],
        lnz[0:1, K * BS:2 * K * BS].rearrange("p (k b) -> p b k", b=BS),
        axis=mybir.AxisListType.X, op=mybir.AluOpType.add)
    red_h = smp.tile([1, BS], F32, tag="rh")
    nc.vector.tensor_reduce(
        red_h[:, :],
        lnz[0:1, BS:K * BS].rearrange("p (k b) -> p b k", b=BS),
        axis=mybir.AxisListType.X, op=mybir.AluOpType.add)
    fwd = smp.tile([1, BS], F32, tag="fwd")
    nc.vector.tensor_tensor(fwd[:, :], red_t[:, :], red_h[:, :],
                            op=mybir.AluOpType.subtract)
    nc.sync.dma_start(fwd_o[:, :], fwd[:, :])

    esum = const.tile([128, 1], F32)
    nc.vector.tensor_reduce(esum[:, :], esums[:, :],
                            axis=mybir.AxisListType.X, op=mybir.AluOpType.add)
    nc.sync.dma_start(esum_o[:, :], esum[:, :])


def build():
    nc = bacc.Bacc("TRN2", target_bir_lowering=False, debug=False)
    ins_spec = [
        ("featT", [128, NCHUNK * CCH], BF16),
        ("transT", [T, T], F32),
        ("stopcol", [T, 1], F32),
        ("initcol", [T, BS], BF16),
        ("table", [128, 256], F32),
        ("counts", [128, 256], BF16),
        ("eidx", [128, NCHUNK * BS], U16),
        ("emitmask", [128, EMW], BF16),
    ]
    outs_spec = [
        ("fwd", [1, BS], F32),
        ("esum", [128, 1], F32),
        ("tsum", [128, 1], F32),
    ]
    ins = [nc.declare_dram_parameter(n, s, d, isOutput=False).ap()
           for n, s, d in ins_spec]
    outs = [nc.declare_dram_parameter(n, s, d, isOutput=True).ap()
            for n, s, d in outs_spec]
    with tile.TileContext(nc) as tc:
        with ExitStack() as ctx:
            crf_kernel(ctx, tc, outs, ins)
    nc.compile()
    return nc


def host_prep(feats, transitions, tags, mask):
    """Build the 8 per-core input maps + host-side pad corrections."""
    assert feats.shape == (B, S, T) and transitions.shape == (T, T)
    mask_arr = np.asarray(mask)
    assert np.all(mask_arr == 1), "kernel assumes an all-ones mask"
    feats = np.asarray(feats, dtype=np.float32)
    transitions = np.asarray(transitions, dtype=np.float32)
    tags = np.asarray(tags).astype(np.int64)

    transT = np.ascontiguousarray(transitions.T)
    stopcol = np.ascontiguousarray(transitions[STOP_TAG, :].reshape(T, 1))
    initcol = np.zeros((T, BS), ml_dtypes.bfloat16)
    initcol[START_TAG, :] = 1.0
    tflat = transitions.reshape(4096)
    table = np.zeros((128, 256), np.float32)
    p_ = np.arange(128)
    table[:, :] = tflat[(p_[:, None] % 16) + 16 * np.arange(256)[None, :]]

    emitmask = np.zeros((128, EMW), ml_dtypes.bfloat16)
    i_ = np.arange(EMW)[None, :]
    emitmask[(p_[:, None] % 16) == (i_ % 16)] = 1.0

    in_maps, corrs = [], []
    for core in range(NCORES):
        b0 = core * BS
        fb = feats[b0:b0 + BS].astype(ml_dtypes.bfloat16)   # (64, 1024, 64)
        tg = tags[b0:b0 + BS]

        # featT[(s%2)*64+n, c*2048 + ((s%64)//2)*64 + b] = fb[b, s, n]
        x = fb.reshape(BS, K, L // 2, 2, T)                 # (b, c, j, hs, n)
        featT = np.ascontiguousarray(
            x.transpose(3, 4, 1, 2, 0).reshape(128, NCHUNK * CCH))

        # emit gather indices: per (chunk, partition-group) lane-bucketed
        eidx = np.zeros((128, NCHUNK * BS), np.uint16)
        corr = 0.0
        featT_f = featT.astype(np.float32)
        for c in range(NCHUNK):
            buckets = [[[] for _ in range(16)] for _ in range(8)]
            for b in range(BS):
                for si in range(L):
                    s = c * L + si
                    t = int(tg[b, s])
                    buckets[(s % 2) * 4 + t // 16][t % 16].append(
                        (si // 2) * BS + b)
            for gidx in range(8):
                for lane in range(16):
                    lst = buckets[gidx][lane]
                    prow = 16 * gidx + lane
                    npad = BS - len(lst)
                    assert npad >= 0, "lane bucket overflow"
                    corr += npad * float(featT_f[prow, c * CCH])
                    for pos, col in enumerate(lst):
                        eidx[prow, c * BS + pos] = col
        corrs.append(corr)

        # transition-pair histogram (bf16-exact small counts)
        cur = np.concatenate([tg, np.full((BS, 1), STOP_TAG, np.int64)], 1)
        prev = np.concatenate([np.full((BS, 1), START_TAG, np.int64), tg], 1)
        lin = (cur * T + prev).reshape(-1)
        cnt = np.bincount(lin, minlength=4096)
        assert cnt.max() < 256
        counts = np.zeros((128, 256), ml_dtypes.bfloat16)
        counts[0:16, :] = cnt.reshape(256, 16).T

        in_maps.append({
            "featT": featT, "transT": transT, "stopcol": stopcol,
            "initcol": initcol, "table": table, "counts": counts,
            "eidx": eidx, "emitmask": emitmask,
        })
    return in_maps, corrs


def host_finish(results, corrs):
    fwd_total = 0.0
    gold_total = 0.0
    for core, r in enumerate(results):
        fwd_total += float(r["fwd"].astype(np.float64).sum()) + BS * S * CSHIFT
        gold_total += float(r["esum"].astype(np.float64).sum()) - corrs[core]
        gold_total += float(r["tsum"].astype(np.float64).sum())
    return np.asarray((fwd_total - gold_total) / B, dtype=np.float32)


_NC = None


def kernel(feats, transitions, tags, mask):
    global _NC
    if _NC is None:
        _NC = build()
    in_maps, corrs = host_prep(feats, transitions, tags, mask)
    res = run_bass_kernel_spmd(_NC, in_maps, list(range(NCORES)))
    return host_finish(res.results, corrs)


if __name__ == "__main__":
    import reference
    inp = reference.setup_inputs()
    out = kernel(**{k: np.asarray(v) for k, v in inp.items()})
    print("kernel loss:", out)


# revision 33
# speedup vs baseline: 4.2479x; 1.1797x over previous
"""CRF loss (forward-algorithm partition function minus gold score, batch mean)
on 8 Trainium2 NeuronCores.

Strategy: pure data parallel over batch (512 -> 64 per core), plus a 16-way
SEQUENCE split per core to break the serial recurrence.

Per-core math (exp-space reformulation of the log-space recurrence):
    e_{s+1} = exp(feat_s) * (M @ e_s),   M[n,p] = exp(trans[n,p] - c)
Products of positive matrices contract to rank-1 exponentially fast, so the
1024-step chain is split into 16 segments of 64 steps. Each segment's chain
starts DELTA=8 steps early from a uniform vector ("warmup"): after 8 steps
the state direction matches the true forward message to ~1e-6 (validated in
numpy); only the scale is off. Scales are stitched with column-sum ratios:
    log z = log(stop . u_15) + sum_k [log t_{k-1} - log h_k]
where t_k = colsum of chain k's final state and h_k = colsum of chain k's
state at warmup end. Chain 0 needs no warmup (it is reset to the exact
e_start at round DELTA).

The 16 chains run as 4 independent LANES (2 gangs x {DVE, Pool}): per round
each lane does one bf16 matmul (PE) + one elementwise multiply (its engine).
Lanes have separate state/psum tiles so the Tile framework inserts no
cross-lane hazards; the round-critical path of each lane hides behind the
other lanes' engine work. exp(feat) runs on the Activation engine in-loop
over 4-round blocks, reading a host-pre-transposed bf16 feats buffer
(tag-major), so there are no device-side DMA transposes.

Gold score:
  - emit: per chunk, one gpsimd indirect_copy gathers feats[b,s,tag] from the
    tag-major feats buffer with group-shared, lane-bucketed indices + a
    periodic 0/1 mask, reduced by one DVE tensor_tensor_reduce per chunk.
    Pad slots point at slice col 0 and are subtracted on the host (exact,
    computed from the same bf16 values the device gathers).
  - transitions: sum trans[cur,prev] = <histogram(cur*64+prev), trans> — the
    host counts index pairs (pure index prep), the device does one
    tensor_tensor_reduce dot product against the replicated table.
"""

import numpy as np
import ml_dtypes
from contextlib import ExitStack

import concourse.bass as bass
import concourse.tile as tile
from concourse import bacc, mybir
from concourse.bass_utils import run_bass_kernel_spmd

F32 = mybir.dt.float32
F8 = mybir.dt.float8e4
BF16 = mybir.dt.bfloat16
U16 = mybir.dt.uint16

B, S, T = 512, 1024, 64
NCORES = 8
BS = B // NCORES          # 64 batches per core
START_TAG, STOP_TAG = 62, 63
CSHIFT = 5.1              # per-step constant log shift folded into M

K = 16                    # sequence segments (= 64-step chunks)
L = S // K                # 64 steps per segment
DELTA = 4                 # warmup steps per chain (multiple of 4)
R = L + DELTA             # 72 rounds
G = 8                     # chains per gang
NG = K // G               # 2 gangs
GW = G * BS               # 512 cols per gang
CCH = (L // 2) * BS       # 2048 cols per chunk in featT
NCHUNK = K
CH_DVE = 8                # chains per gang multiplied on DVE (all of them)
EMW = 16 * BS             # 1024: emit gather stream width per chunk


def crf_kernel(ctx: ExitStack, tc: tile.TileContext, outs, ins):
    nc = tc.nc
    (fwd_o, esum_o, tsum_o) = outs
    (featT_i, transT_i, stopcol_i, initcol_i, table_i, counts_i,
     eidx_i, emitmask_i) = ins

    const = ctx.enter_context(tc.tile_pool(name="const", bufs=1))
    efp = ctx.enter_context(tc.tile_pool(name="ef", bufs=3))
    egp = ctx.enter_context(tc.tile_pool(name="eg", bufs=2))
    scr = ctx.enter_context(tc.tile_pool(name="scr", bufs=2))
    qp = [ctx.enter_context(tc.tile_pool(name=f"q{g}", bufs=2, space="PSUM"))
          for g in range(NG)]
    zp = ctx.enter_context(tc.tile_pool(name="z", bufs=1, space="PSUM"))
    smp = ctx.enter_context(tc.tile_pool(name="sm", bufs=2))

    # ---- constants ----
    mtraw = const.tile([128, T], F32)
    nc.sync.dma_start(mtraw[0:64, :], transT_i[:, :])
    nc.sync.dma_start(mtraw[64:128, :], transT_i[:, :])
    negc = const.tile([128, 1], F32)
    nc.vector.memset(negc[:, :], -CSHIFT)
    mt = const.tile([128, T], BF16)   # exp(trans.T - c), both halves, bf16
    nc.scalar.activation(mt[:, :], mtraw[:, :],
                         mybir.ActivationFunctionType.Exp, bias=negc[:, :])

    stopraw = const.tile([128, 1], F32)
    nc.sync.dma_start(stopraw[64:128, :], stopcol_i[:, :])
    stopt = const.tile([128, 1], BF16)
    nc.scalar.activation(stopt[64:128, :], stopraw[64:128, :],
                         mybir.ActivationFunctionType.Exp)

    ones_col = const.tile([128, 1], BF16)
    nc.vector.memset(ones_col[:, :], 1.0)
    initcol = const.tile([128, BS], BF16)  # e_start pattern in rows 64:128
    nc.sync.dma_start(initcol[64:128, :], initcol_i[:, :])

    table = const.tile([128, 256], F32)
    nc.sync.dma_start(table[:, :], table_i[:, :])
    counts = const.tile([128, 256], BF16)
    nc.sync.dma_start(counts[:, :], counts_i[:, :])
    eidx = const.tile([128, NCHUNK * BS], U16)
    nc.sync.dma_start(eidx[:, :], eidx_i[:, :])
    emitmask = const.tile([128, EMW], BF16)
    nc.sync.dma_start(emitmask[:, :], emitmask_i[:, :])

    # gold transitions: one dot product against the replicated table
    tsc = const.tile([128, 256], F32)
    tsum = const.tile([128, 1], F32)
    nc.vector.scalar_tensor_tensor(tsc[:, :], table[:, :], 1.0, counts[:, :],
                                   op0=mybir.AluOpType.mult,
                                   op1=mybir.AluOpType.mult,
                                   accum_out=tsum[:, :])
    nc.sync.dma_start(tsum_o[:, :], tsum[:, :])

    # ---- feats (tag-major bf16, host-transposed), streamed per chunk ----
    featT = const.tile([128, (NCHUNK + 1) * CCH], F8)
    nc.vector.memset(featT[:, 0:CCH], 0.0)           # warmup chunk: feat = 0
    for c in range(1, NCHUNK + 1):
        nc.sync.dma_start(featT[:, c * CCH:(c + 1) * CCH],
                          featT_i[:, (c - 1) * CCH:c * CCH])
    featT3 = featT[:, :].rearrange("p (c x) -> p c x", x=CCH)

    # ---- per-gang state [128, 512]; one DVE multiply per gang per round
    #      (DVE is the only engine that can both read PSUM and do tensor x
    #      tensor; the two gangs interleave to hide each other's latency).
    sts = []
    for g in range(NG):
        st = const.tile([128, GW], BF16, tag=f"st{g}")
        nc.vector.memset(st[0:64, :], 1.0 / T)
        nc.vector.memset(st[64:128, :], 1.0 / T)
        sts.append(st)

    # log-scale stash: [h(16 chains) | t(16 chains)] x 64 batches
    zbuf = const.tile([1, 2 * K * BS], F32)

    esums = const.tile([128, NCHUNK], F32)

    NBLK = R // 4
    ef_blks = [[None] * NBLK for _ in range(NG)]

    def issue_exp(g, m):
        # ef block (4 rounds): exp of the 8 chains' 2-column (128-elem) window
        eb = efp.tile([128, G * 2 * BS], BF16, tag=f"ef{g}_{m % 4}")
        r0 = 4 * m
        off = 0 if r0 < DELTA else 1
        j0 = (L - DELTA + r0) // 2 if r0 < DELTA else (r0 - DELTA) // 2
        i0 = G * g + off
        src3 = featT3[:, i0:i0 + G, j0 * BS:(j0 + 2) * BS]
        nc.scalar.activation(eb[:, :].rearrange("p (i x) -> p i x", x=2 * BS),
                             src3, mybir.ActivationFunctionType.Exp)
        ef_blks[g][m] = eb

    for g in range(NG):
        issue_exp(g, 0)
        issue_exp(g, 1)
        issue_exp(g, 2)

    def issue_emit(c):
        # gather (Pool) -> periodic-mask multiply (Pool) -> accumulate (Act)
        eg = egp.tile([128, EMW], F8)
        nc.gpsimd.indirect_copy(eg[:, :],
                                featT[:, (c + 1) * CCH:(c + 2) * CCH],
                                eidx[:, c * BS:(c + 1) * BS], True)
        sc = scr.tile([128, EMW], BF16)
        nc.gpsimd.tensor_tensor(sc[:, :], eg[:, :], emitmask[:, :],
                                op=mybir.AluOpType.mult)
        nc.scalar.activation(sc[:, :], sc[:, :],
                             mybir.ActivationFunctionType.Copy,
                             accum_out=esums[:, c:c + 1])

    for c in range(6):
        issue_emit(c)
    emit_next = 6

    def capture(dst_off):
        # column-sums of all 16 chains' states into zbuf[dst_off:dst_off+1024]
        for g in range(NG):
            z = zp.tile([1, GW], F32, tag=f"z{g}")
            nc.tensor.matmul(z[:, :], ones_col[64:128, :],
                             sts[g][64:128, :], tile_position=(64, 0))
            nc.vector.tensor_copy(
                zbuf[0:1, dst_off + g * GW:dst_off + (g + 1) * GW], z[:, :])

    # ---- main loop ----
    # Round-major pacing hint: without it the Tile scheduler lets the
    # fast (Pool) lanes run many rounds ahead of the DVE lanes, and the
    # in-order PE queue then head-of-line blocks ready matmuls behind
    # stalled ones.  PACE_US is slightly above the per-round engine floor.
    PACE_US = 0.00120
    HEAD_US = 0.0
    for r in range(R):
        tc.tile_set_cur_wait(HEAD_US + r * PACE_US)
        hs = r % 2
        hr = 1 - hs
        m, jj = r // 4, (r % 4) // 2
        if r % 4 == 0 and m + 3 < NBLK:
            for g in range(NG):
                issue_exp(g, m + 3)
        if r % 4 == 2 and emit_next < NCHUNK:
            issue_emit(emit_next)
            emit_next += 1
        if r == DELTA:
            # h-capture at position 64k (state after round DELTA-1, half 1)
            capture(0)
            # chain-0 reset to the exact e_start (gang 0, chain 0)
            nc.scalar.copy(sts[0][64:128, 0:BS], initcol[64:128, :])
        for g in range(NG):
            st = sts[g]
            eb = ef_blks[g][m]
            ebv = eb[hs * 64:hs * 64 + 64, :].rearrange(
                "p (i y b) -> p i y b", i=G, y=2)[:, :, jj, :]
            q = qp[g].tile([128, GW], F32)
            nc.tensor.matmul(q[hs * 64:hs * 64 + 64, :],
                             mt[hr * 64:hr * 64 + 64, :],
                             st[hr * 64:hr * 64 + 64, :],
                             tile_position=(hr * 64, hs * 64))
            nc.vector.tensor_tensor(
                st[hs * 64:hs * 64 + 64, :].rearrange(
                    "p (i b) -> p i b", b=BS),
                q[hs * 64:hs * 64 + 64, :].rearrange(
                    "p (i b) -> p i b", b=BS),
                ebv[:, :, :],
                op=mybir.AluOpType.mult)

    while emit_next < NCHUNK:
        issue_emit(emit_next)
        emit_next += 1

    # ---- t-capture (final states live in half 1 after round R-1) ----
    capture(K * BS)
    # chain 15 terminal: stop . state overwrites its t slot
    tq = zp.tile([1, BS], F32, tag="tq")
    nc.tensor.matmul(tq[:, :], stopt[64:128, :],
                     sts[1][64:128, GW - BS:GW],
                     tile_position=(64, 0))
    nc.vector.tensor_copy(zbuf[0:1, 2 * K * BS - BS:2 * K * BS], tq[:, :])

    # ---- fwd[b] = sum_k ln t'_k[b] - sum_{k>=1} ln h_k[b] ----
    lnz = smp.tile([1, 2 * K * BS], F32, tag="lnz")
    nc.scalar.activation(lnz[:, :], zbuf[:, :],
                         mybir.ActivationFunctionType.Ln)
    red_t = smp.tile([1, BS], F32, tag="rt")
    nc.vector.tensor_reduce(
        red_t[:, :],
        lnz[0:1, K * BS:2 * K * BS].rearrange("p (k b) -> p b k", b=BS),
        axis=mybir.AxisListType.X, op=mybir.AluOpType.add)
    red_h = smp.tile([1, BS], F32, tag="rh")
    nc.vector.tensor_reduce(
        red_h[:, :],
        lnz[0:1, BS:K * BS].rearrange("p (k b) -> p b k", b=BS),
        axis=mybir.AxisListType.X, op=mybir.AluOpType.add)
    fwd = smp.tile([1, BS], F32, tag="fwd")
    nc.vector.tensor_tensor(fwd[:, :], red_t[:, :], red_h[:, :],
                            op=mybir.AluOpType.subtract)
    nc.sync.dma_start(fwd_o[:, :], fwd[:, :])

    esum = const.tile([128, 1], F32)
    nc.vector.tensor_reduce(esum[:, :], esums[:, :],
                            axis=mybir.AxisListType.X, op=mybir.AluOpType.add)
    nc.sync.dma_start(esum_o[:, :], esum[:, :])


def build():
    nc = bacc.Bacc("TRN2", target_bir_lowering=False, debug=False)
    ins_spec = [
        ("featT", [128, NCHUNK * CCH], F8),
        ("transT", [T, T], F32),
        ("stopcol", [T, 1], F32),
        ("initcol", [T, BS], BF16),
        ("table", [128, 256], F32),
        ("counts", [128, 256], BF16),
        ("eidx", [128, NCHUNK * BS], U16),
        ("emitmask", [128, EMW], BF16),
    ]
    outs_spec = [
        ("fwd", [1, BS], F32),
        ("esum", [128, 1], F32),
        ("tsum", [128, 1], F32),
    ]
    ins = [nc.declare_dram_parameter(n, s, d, isOutput=False).ap()
           for n, s, d in ins_spec]
    outs = [nc.declare_dram_parameter(n, s, d, isOutput=True).ap()
            for n, s, d in outs_spec]
    with tile.TileContext(nc) as tc:
        with ExitStack() as ctx:
            crf_kernel(ctx, tc, outs, ins)
    nc.compile()
    return nc


def host_prep(feats, transitions, tags, mask):
    """Build the 8 per-core input maps + host-side pad corrections."""
    assert feats.shape == (B, S, T) and transitions.shape == (T, T)
    mask_arr = np.asarray(mask)
    assert np.all(mask_arr == 1), "kernel assumes an all-ones mask"
    feats = np.asarray(feats, dtype=np.float32)
    transitions = np.asarray(transitions, dtype=np.float32)
    tags = np.asarray(tags).astype(np.int64)

    transT = np.ascontiguousarray(transitions.T)
    stopcol = np.ascontiguousarray(transitions[STOP_TAG, :].reshape(T, 1))
    initcol = np.zeros((T, BS), ml_dtypes.bfloat16)
    initcol[START_TAG, :] = 1.0
    tflat = transitions.reshape(4096)
    table = np.zeros((128, 256), np.float32)
    p_ = np.arange(128)
    table[:, :] = tflat[(p_[:, None] % 16) + 16 * np.arange(256)[None, :]]

    emitmask = np.zeros((128, EMW), ml_dtypes.bfloat16)
    i_ = np.arange(EMW)[None, :]
    emitmask[(p_[:, None] % 16) == (i_ % 16)] = 1.0

    in_maps, corrs = [], []
    for core in range(NCORES):
        b0 = core * BS
        fb = feats[b0:b0 + BS].astype(ml_dtypes.float8_e4m3fn)  # (64,1024,64)
        tg = tags[b0:b0 + BS]

        # featT[(s%2)*64+n, c*2048 + ((s%64)//2)*64 + b] = fb[b, s, n]
        x = fb.reshape(BS, K, L // 2, 2, T)                 # (b, c, j, hs, n)
        featT = np.ascontiguousarray(
            x.transpose(3, 4, 1, 2, 0).reshape(128, NCHUNK * CCH))

        # emit gather indices: per (chunk, partition-group) lane-bucketed
        eidx = np.zeros((128, NCHUNK * BS), np.uint16)
        corr = 0.0
        featT_f = featT.astype(np.float32)
        for c in range(NCHUNK):
            buckets = [[[] for _ in range(16)] for _ in range(8)]
            for b in range(BS):
                for si in range(L):
                    s = c * L + si
                    t = int(tg[b, s])
                    buckets[(s % 2) * 4 + t // 16][t % 16].append(
                        (si // 2) * BS + b)
            for gidx in range(8):
                for lane in range(16):
                    lst = buckets[gidx][lane]
                    prow = 16 * gidx + lane
                    npad = BS - len(lst)
                    assert npad >= 0, "lane bucket overflow"
                    corr += npad * float(featT_f[prow, c * CCH])
                    for pos, col in enumerate(lst):
                        eidx[prow, c * BS + pos] = col
        corrs.append(corr)

        # transition-pair histogram (bf16-exact small counts)
        cur = np.concatenate([tg, np.full((BS, 1), STOP_TAG, np.int64)], 1)
        prev = np.concatenate([np.full((BS, 1), START_TAG, np.int64), tg], 1)
        lin = (cur * T + prev).reshape(-1)
        cnt = np.bincount(lin, minlength=4096)
        assert cnt.max() < 256
        counts = np.zeros((128, 256), ml_dtypes.bfloat16)
        counts[0:16, :] = cnt.reshape(256, 16).T

        in_maps.append({
            "featT": featT, "transT": transT, "stopcol": stopcol,
            "initcol": initcol, "table": table, "counts": counts,
            "eidx": eidx, "emitmask": emitmask,
        })
    return in_maps, corrs


def host_finish(results, corrs):
    fwd_total = 0.0
    gold_total = 0.0
    for core, r in enumerate(results):
        fwd_total += float(r["fwd"].astype(np.float64).sum()) + BS * S * CSHIFT
        gold_total += float(r["esum"].astype(np.float64).sum()) - corrs[core]
        gold_total += float(r["tsum"].astype(np.float64).sum())
    return np.asarray((fwd_total - gold_total) / B, dtype=np.float32)


_NC = None


def kernel(feats, transitions, tags, mask):
    global _NC
    if _NC is None:
        _NC = build()
    in_maps, corrs = host_prep(feats, transitions, tags, mask)
    res = run_bass_kernel_spmd(_NC, in_maps, list(range(NCORES)))
    return host_finish(res.results, corrs)


if __name__ == "__main__":
    import reference
    inp = reference.setup_inputs()
    out = kernel(**{k: np.asarray(v) for k, v in inp.items()})
    print("kernel loss:", out)


# revision 39
# speedup vs baseline: 4.8047x; 1.1311x over previous
"""CRF loss (forward-algorithm partition function minus gold score, batch mean)
on 8 Trainium2 NeuronCores.

Strategy: pure data parallel over batch (512 -> 64 per core), plus a 16-way
SEQUENCE split per core to break the serial recurrence.

Per-core math (exp-space reformulation of the log-space recurrence):
    e_{s+1} = exp(feat_s) * (M @ e_s),   M[n,p] = exp(trans[n,p] - c)
Products of positive matrices contract to rank-1 exponentially fast, so the
1024-step chain is split into 16 segments of 64 steps. Each segment's chain
starts DELTA=8 steps early from a uniform vector ("warmup"): after 8 steps
the state direction matches the true forward message to ~1e-6 (validated in
numpy); only the scale is off. Scales are stitched with column-sum ratios:
    log z = log(stop . u_15) + sum_k [log t_{k-1} - log h_k]
where t_k = colsum of chain k's final state and h_k = colsum of chain k's
state at warmup end. Chain 0 needs no warmup (it is reset to the exact
e_start at round DELTA).

The 16 chains run as 4 independent LANES (2 gangs x {DVE, Pool}): per round
each lane does one bf16 matmul (PE) + one elementwise multiply (its engine).
Lanes have separate state/psum tiles so the Tile framework inserts no
cross-lane hazards; the round-critical path of each lane hides behind the
other lanes' engine work. exp(feat) runs on the Activation engine in-loop
over 4-round blocks, reading a host-pre-transposed bf16 feats buffer
(tag-major), so there are no device-side DMA transposes.

Gold score:
  - emit: per chunk, one gpsimd indirect_copy gathers feats[b,s,tag] from the
    tag-major feats buffer with group-shared, lane-bucketed indices + a
    periodic 0/1 mask, reduced by one DVE tensor_tensor_reduce per chunk.
    Pad slots point at slice col 0 and are subtracted on the host (exact,
    computed from the same bf16 values the device gathers).
  - transitions: sum trans[cur,prev] = <histogram(cur*64+prev), trans> — the
    host counts index pairs (pure index prep), the device does one
    tensor_tensor_reduce dot product against the replicated table.
"""

import numpy as np
import ml_dtypes
from contextlib import ExitStack

import concourse.bass as bass
import concourse.tile as tile
from concourse import bacc, mybir
from concourse.bass_utils import run_bass_kernel_spmd

F32 = mybir.dt.float32
F8 = mybir.dt.float8e4
BF16 = mybir.dt.bfloat16
U16 = mybir.dt.uint16

B, S, T = 512, 1024, 64
NCORES = 8
BS = B // NCORES          # 64 batches per core
START_TAG, STOP_TAG = 62, 63
CSHIFT = 5.1              # per-step constant log shift folded into M

K = 16                    # sequence segments (= 64-step chunks)
L = S // K                # 64 steps per segment
DELTA = 4                 # warmup steps per chain (multiple of 4)
R = L + DELTA             # 72 rounds
G = 8                     # chains per gang
NG = K // G               # 2 gangs
GW = G * BS               # 512 cols per gang
CCH = (L // 2) * BS       # 2048 cols per chunk in featT
NCHUNK = K
CH_DVE = 8                # chains per gang multiplied on DVE (all of them)
EMW = 16 * BS             # 1024: emit gather stream width per chunk


def crf_kernel(ctx: ExitStack, tc: tile.TileContext, outs, ins):
    nc = tc.nc
    (fwd_o, esum_o, tsum_o) = outs
    (featT_i, transT_i, stopcol_i, initcol_i, table_i, counts_i,
     oh8_i, eye_i) = ins

    const = ctx.enter_context(tc.tile_pool(name="const", bufs=1))
    efp = ctx.enter_context(tc.tile_pool(name="ef", bufs=3))
    qp = [ctx.enter_context(tc.tile_pool(name=f"q{g}", bufs=2, space="PSUM"))
          for g in range(NG)]
    zp = ctx.enter_context(tc.tile_pool(name="z", bufs=1, space="PSUM"))
    dqp = ctx.enter_context(tc.tile_pool(name="dq", bufs=1, space="PSUM"))
    smp = ctx.enter_context(tc.tile_pool(name="sm", bufs=2))

    # ---- constants ----
    mtraw = const.tile([128, T], F32)
    nc.sync.dma_start(mtraw[0:64, :], transT_i[:, :])
    nc.sync.dma_start(mtraw[64:128, :], transT_i[:, :])
    negc = const.tile([128, 1], F32)
    nc.vector.memset(negc[:, :], -CSHIFT)
    mt = const.tile([128, T], BF16)   # exp(trans.T - c), both halves, bf16
    nc.scalar.activation(mt[:, :], mtraw[:, :],
                         mybir.ActivationFunctionType.Exp, bias=negc[:, :])

    stopraw = const.tile([128, 1], F32)
    nc.sync.dma_start(stopraw[64:128, :], stopcol_i[:, :])
    stopt = const.tile([128, 1], BF16)
    nc.scalar.activation(stopt[64:128, :], stopraw[64:128, :],
                         mybir.ActivationFunctionType.Exp)

    ones_col = const.tile([128, 1], BF16)
    nc.vector.memset(ones_col[:, :], 1.0)
    initcol = const.tile([128, BS], BF16)  # e_start pattern in rows 64:128
    nc.sync.dma_start(initcol[64:128, :], initcol_i[:, :])

    table = const.tile([128, 256], F32)
    nc.sync.dma_start(table[:, :], table_i[:, :])
    counts = const.tile([128, 256], BF16)
    nc.sync.dma_start(counts[:, :], counts_i[:, :])
    eye = const.tile([T, T], BF16)
    nc.sync.dma_start(eye[:, :], eye_i[:, :])

    # gold transitions: one dot product against the replicated table
    tsc = const.tile([128, 256], F32)
    tsum = const.tile([128, 1], F32)
    nc.vector.scalar_tensor_tensor(tsc[:, :], table[:, :], 1.0, counts[:, :],
                                   op0=mybir.AluOpType.mult,
                                   op1=mybir.AluOpType.mult,
                                   accum_out=tsum[:, :])
    nc.sync.dma_start(tsum_o[:, :], tsum[:, :])

    # ---- feats (tag-major bf16, host-transposed), streamed per chunk ----
    featT = const.tile([128, (NCHUNK + 1) * CCH], F8)
    nc.vector.memset(featT[:, 0:CCH], 0.0)           # warmup chunk: feat = 0
    for c in range(1, NCHUNK + 1):
        nc.sync.dma_start(featT[:, c * CCH:(c + 1) * CCH],
                          featT_i[:, (c - 1) * CCH:c * CCH])
    featT3 = featT[:, :].rearrange("p (c x) -> p c x", x=CCH)

    oh8 = const.tile([128, NCHUNK * CCH], F8)
    for c in range(NCHUNK):
        nc.sync.dma_start(oh8[:, c * CCH:(c + 1) * CCH],
                          oh8_i[:, c * CCH:(c + 1) * CCH])

    # ---- per-gang state [128, 512]; one DVE multiply per gang per round
    #      (DVE is the only engine that can both read PSUM and do tensor x
    #      tensor; the two gangs interleave to hide each other's latency).
    sts = []
    for g in range(NG):
        st = const.tile([128, GW], BF16, tag=f"st{g}")
        nc.vector.memset(st[0:64, :], 1.0 / T)
        nc.vector.memset(st[64:128, :], 1.0 / T)
        sts.append(st)

    # log-scale stash: [h(16 chains) | t(16 chains)] x 64 batches
    zbuf = const.tile([1, 2 * K * BS], F32)

    esums = const.tile([128, NCHUNK], F32)
    nc.vector.memset(esums[:, :], 0.0)

    NBLK = R // 4
    ef_blks = [[None] * NBLK for _ in range(NG)]

    def issue_exp(g, m):
        # ef block (4 rounds): exp of the 8 chains' 2-column (128-elem) window
        eb = efp.tile([128, G * 2 * BS], BF16, tag=f"ef{g}_{m % 4}")
        r0 = 4 * m
        off = 0 if r0 < DELTA else 1
        j0 = (L - DELTA + r0) // 2 if r0 < DELTA else (r0 - DELTA) // 2
        i0 = G * g + off
        src3 = featT3[:, i0:i0 + G, j0 * BS:(j0 + 2) * BS]
        nc.scalar.activation(eb[:, :].rearrange("p (i x) -> p i x", x=2 * BS),
                             src3, mybir.ActivationFunctionType.Exp)
        ef_blks[g][m] = eb

    for g in range(NG):
        issue_exp(g, 0)
        issue_exp(g, 1)
        issue_exp(g, 2)

    dq_cur = [None, None]

    def issue_emit_part(c, part):
        # emit via one-hot diag matmuls: accumulating fp8 matmuls, then a
        # diag-extract. One accumulation group per step-parity: the PE
        # cannot alternate tile_position quadrants within a group.
        if part == 0:
            dq_e = dqp.tile([64, T], F32, tag="dqe")
            dq_o = dqp.tile([64, T], F32, tag="dqo")
            dq_cur[0], dq_cur[1] = dq_e, dq_o
        for k in range(16):
            s_in = part * 16 + k
            hs, j = s_in % 2, s_in // 2
            col0 = (c + 1) * CCH + j * BS
            ohcol0 = c * CCH + j * BS
            nc.tensor.matmul(dq_cur[hs][:, :],
                             featT[hs * 64:hs * 64 + 64, col0:col0 + BS],
                             oh8[hs * 64:hs * 64 + 64, ohcol0:ohcol0 + BS],
                             start=(s_in == hs), stop=(s_in >= 62),
                             tile_position=(hs * 64, 0))
        if part == 3:
            for hs in (0, 1):
                dsc = smp.tile([64, T], BF16, tag=f"dsc{hs}")
                nc.vector.scalar_tensor_tensor(
                    dsc[:, :], dq_cur[hs][:, :], 1.0, eye[:, :],
                    op0=mybir.AluOpType.mult, op1=mybir.AluOpType.mult,
                    accum_out=esums[hs * 64:hs * 64 + 64, c:c + 1])

    def capture(dst_off):
        # column-sums of all 16 chains' states into zbuf[dst_off:dst_off+1024]
        for g in range(NG):
            z = zp.tile([1, GW], F32, tag=f"z{g}")
            nc.tensor.matmul(z[:, :], ones_col[64:128, :],
                             sts[g][64:128, :], tile_position=(64, 0))
            nc.vector.tensor_copy(
                zbuf[0:1, dst_off + g * GW:dst_off + (g + 1) * GW], z[:, :])

    # ---- main loop ----
    # Round-major pacing hint: without it the Tile scheduler lets the
    # fast (Pool) lanes run many rounds ahead of the DVE lanes, and the
    # in-order PE queue then head-of-line blocks ready matmuls behind
    # stalled ones.  PACE_US is slightly above the per-round engine floor.
    PACE_US = 0.00120
    HEAD_US = 0.0
    for r in range(R):
        tc.tile_set_cur_wait(HEAD_US + r * PACE_US)
        hs = r % 2
        hr = 1 - hs
        m, jj = r // 4, (r % 4) // 2
        if r % 4 == 0 and m + 3 < NBLK:
            for g in range(NG):
                issue_exp(g, m + 3)
        if r >= 2 and (r - 2) // 4 < NCHUNK:
            issue_emit_part((r - 2) // 4, (r - 2) % 4)
        if r == DELTA:
            # h-capture at position 64k (state after round DELTA-1, half 1)
            capture(0)
            # chain-0 reset to the exact e_start (gang 0, chain 0)
            nc.scalar.copy(sts[0][64:128, 0:BS], initcol[64:128, :])
        for g in range(NG):
            st = sts[g]
            eb = ef_blks[g][m]
            ebv = eb[hs * 64:hs * 64 + 64, :].rearrange(
                "p (i y b) -> p i y b", i=G, y=2)[:, :, jj, :]
            q = qp[g].tile([128, GW], F32)
            nc.tensor.matmul(q[hs * 64:hs * 64 + 64, :],
                             mt[hr * 64:hr * 64 + 64, :],
                             st[hr * 64:hr * 64 + 64, :],
                             tile_position=(hr * 64, hs * 64))
            nc.vector.tensor_tensor(
                st[hs * 64:hs * 64 + 64, :].rearrange(
                    "p (i b) -> p i b", b=BS),
                q[hs * 64:hs * 64 + 64, :].rearrange(
                    "p (i b) -> p i b", b=BS),
                ebv[:, :, :],
                op=mybir.AluOpType.mult)

    # ---- t-capture (final states live in half 1 after round R-1) ----
    capture(K * BS)
    # chain 15 terminal: stop . state overwrites its t slot (reuses z1's bank)
    tq = zp.tile([1, GW], F32, tag="z1")
    nc.tensor.matmul(tq[0:1, 0:BS], stopt[64:128, :],
                     sts[1][64:128, GW - BS:GW],
                     tile_position=(64, 0))
    nc.vector.tensor_copy(zbuf[0:1, 2 * K * BS - BS:2 * K * BS],
                          tq[0:1, 0:BS])

    # ---- fwd[b] = sum_k ln t'_k[b] - sum_{k>=1} ln h_k[b] ----
    lnz = smp.tile([1, 2 * K * BS], F32, tag="lnz")
    nc.scalar.activation(lnz[:, :], zbuf[:, :],
                         mybir.ActivationFunctionType.Ln)
    red_t = smp.tile([1, BS], F32, tag="rt")
    nc.vector.tensor_reduce(
        red_t[:, :],
        lnz[0:1, K * BS:2 * K * BS].rearrange("p (k b) -> p b k", b=BS),
        axis=mybir.AxisListType.X, op=mybir.AluOpType.add)
    red_h = smp.tile([1, BS], F32, tag="rh")
    nc.vector.tensor_reduce(
        red_h[:, :],
        lnz[0:1, BS:K * BS].rearrange("p (k b) -> p b k", b=BS),
        axis=mybir.AxisListType.X, op=mybir.AluOpType.add)
    fwd = smp.tile([1, BS], F32, tag="fwd")
    nc.vector.tensor_tensor(fwd[:, :], red_t[:, :], red_h[:, :],
                            op=mybir.AluOpType.subtract)
    nc.sync.dma_start(fwd_o[:, :], fwd[:, :])

    esum = const.tile([128, 1], F32)
    nc.vector.tensor_reduce(esum[:, :], esums[:, :],
                            axis=mybir.AxisListType.X, op=mybir.AluOpType.add)
    nc.sync.dma_start(esum_o[:, :], esum[:, :])


def build():
    nc = bacc.Bacc("TRN2", target_bir_lowering=False, debug=False)
    ins_spec = [
        ("featT", [128, NCHUNK * CCH], F8),
        ("transT", [T, T], F32),
        ("stopcol", [T, 1], F32),
        ("initcol", [T, BS], BF16),
        ("table", [128, 256], F32),
        ("counts", [128, 256], BF16),
        ("oh8", [128, NCHUNK * CCH], F8),
        ("eye", [T, T], BF16),
    ]
    outs_spec = [
        ("fwd", [1, BS], F32),
        ("esum", [128, 1], F32),
        ("tsum", [128, 1], F32),
    ]
    ins = [nc.declare_dram_parameter(n, s, d, isOutput=False).ap()
           for n, s, d in ins_spec]
    outs = [nc.declare_dram_parameter(n, s, d, isOutput=True).ap()
            for n, s, d in outs_spec]
    with tile.TileContext(nc) as tc:
        with ExitStack() as ctx:
            crf_kernel(ctx, tc, outs, ins)
    nc.compile()
    return nc


def host_prep(feats, transitions, tags, mask):
    """Build the 8 per-core input maps + host-side pad corrections."""
    assert feats.shape == (B, S, T) and transitions.shape == (T, T)
    mask_arr = np.asarray(mask)
    assert np.all(mask_arr == 1), "kernel assumes an all-ones mask"
    feats = np.asarray(feats, dtype=np.float32)
    transitions = np.asarray(transitions, dtype=np.float32)
    tags = np.asarray(tags).astype(np.int64)

    transT = np.ascontiguousarray(transitions.T)
    stopcol = np.ascontiguousarray(transitions[STOP_TAG, :].reshape(T, 1))
    initcol = np.zeros((T, BS), ml_dtypes.bfloat16)
    initcol[START_TAG, :] = 1.0
    tflat = transitions.reshape(4096)
    table = np.zeros((128, 256), np.float32)
    p_ = np.arange(128)
    table[:, :] = tflat[(p_[:, None] % 16) + 16 * np.arange(256)[None, :]]

    eye = np.eye(T, dtype=ml_dtypes.bfloat16)

    in_maps = []
    for core in range(NCORES):
        b0 = core * BS
        fb = feats[b0:b0 + BS].astype(ml_dtypes.float8_e4m3fn)  # (64,1024,64)
        tg = tags[b0:b0 + BS]

        # featT[(s%2)*64+n, c*2048 + ((s%64)//2)*64 + b] = fb[b, s, n]
        x = fb.reshape(BS, K, L // 2, 2, T)                 # (b, c, j, hs, n)
        featT = np.ascontiguousarray(
            x.transpose(3, 4, 1, 2, 0).reshape(128, NCHUNK * CCH))

        # one-hot tags in the same tag-major layout (for the emit diag-mms)
        ohsrc = np.zeros((BS, S, T), ml_dtypes.float8_e4m3fn)
        np.put_along_axis(ohsrc, tg[:, :, None], ml_dtypes.float8_e4m3fn(1.0),
                          axis=2)
        xo = ohsrc.reshape(BS, K, L // 2, 2, T)
        oh8 = np.ascontiguousarray(
            xo.transpose(3, 4, 1, 2, 0).reshape(128, NCHUNK * CCH))

        # transition-pair histogram (bf16-exact small counts)
        cur = np.concatenate([tg, np.full((BS, 1), STOP_TAG, np.int64)], 1)
        prev = np.concatenate([np.full((BS, 1), START_TAG, np.int64), tg], 1)
        lin = (cur * T + prev).reshape(-1)
        cnt = np.bincount(lin, minlength=4096)
        assert cnt.max() < 256
        counts = np.zeros((128, 256), ml_dtypes.bfloat16)
        counts[0:16, :] = cnt.reshape(256, 16).T

        in_maps.append({
            "featT": featT, "transT": transT, "stopcol": stopcol,
            "initcol": initcol, "table": table, "counts": counts,
            "oh8": oh8, "eye": eye,
        })
    return in_maps


def host_finish(results):
    fwd_total = 0.0
    gold_total = 0.0
    for r in results:
        fwd_total += float(r["fwd"].astype(np.float64).sum()) + BS * S * CSHIFT
        gold_total += float(r["esum"].astype(np.float64).sum())
        gold_total += float(r["tsum"].astype(np.float64).sum())
    return np.asarray((fwd_total - gold_total) / B, dtype=np.float32)


_NC = None


def kernel(feats, transitions, tags, mask):
    global _NC
    if _NC is None:
        _NC = build()
    in_maps = host_prep(feats, transitions, tags, mask)
    res = run_bass_kernel_spmd(_NC, in_maps, list(range(NCORES)))
    return host_finish(res.results)


if __name__ == "__main__":
    import reference
    inp = reference.setup_inputs()
    out = kernel(**{k: np.asarray(v) for k, v in inp.items()})
    print("kernel loss:", out)


# revision 50
# speedup vs baseline: 4.8866x; 1.0171x over previous
"""CRF loss (forward-algorithm partition function minus gold score, batch mean)
on 8 Trainium2 NeuronCores.

Strategy: pure data parallel over batch (512 -> 64 per core), plus a 16-way
SEQUENCE split per core to break the serial recurrence.

Per-core math (exp-space reformulation of the log-space recurrence):
    e_{s+1} = exp(feat_s) * (M @ e_s),   M[n,p] = exp(trans[n,p] - c)
Products of positive matrices contract to rank-1 exponentially fast, so the
1024-step chain is split into 16 segments of 64 steps. Each segment's chain
starts DELTA=8 steps early from a uniform vector ("warmup"): after 8 steps
the state direction matches the true forward message to ~1e-6 (validated in
numpy); only the scale is off. Scales are stitched with column-sum ratios:
    log z = log(stop . u_15) + sum_k [log t_{k-1} - log h_k]
where t_k = colsum of chain k's final state and h_k = colsum of chain k's
state at warmup end. Chain 0 needs no warmup (it is reset to the exact
e_start at round DELTA).

The 16 chains run as 4 independent LANES (2 gangs x {DVE, Pool}): per round
each lane does one bf16 matmul (PE) + one elementwise multiply (its engine).
Lanes have separate state/psum tiles so the Tile framework inserts no
cross-lane hazards; the round-critical path of each lane hides behind the
other lanes' engine work. exp(feat) runs on the Activation engine in-loop
over 4-round blocks, reading a host-pre-transposed bf16 feats buffer
(tag-major), so there are no device-side DMA transposes.

Gold score:
  - emit: per chunk, one gpsimd indirect_copy gathers feats[b,s,tag] from the
    tag-major feats buffer with group-shared, lane-bucketed indices + a
    periodic 0/1 mask, reduced by one DVE tensor_tensor_reduce per chunk.
    Pad slots point at slice col 0 and are subtracted on the host (exact,
    computed from the same bf16 values the device gathers).
  - transitions: sum trans[cur,prev] = <histogram(cur*64+prev), trans> — the
    host counts index pairs (pure index prep), the device does one
    tensor_tensor_reduce dot product against the replicated table.
"""

import numpy as np
import ml_dtypes
from contextlib import ExitStack

import concourse.bass as bass
import concourse.tile as tile
from concourse import bacc, mybir
from concourse.bass_utils import run_bass_kernel_spmd

F32 = mybir.dt.float32
F8 = mybir.dt.float8e4
BF16 = mybir.dt.bfloat16
U16 = mybir.dt.uint16

B, S, T = 512, 1024, 64
NCORES = 8
BS = B // NCORES          # 64 batches per core
START_TAG, STOP_TAG = 62, 63
CSHIFT = 5.1              # per-step constant log shift folded into M

K = 16                    # sequence segments (= 64-step chunks)
L = S // K                # 64 steps per segment
DELTA = 2                 # warmup steps per chain (even)
R = L + DELTA             # 72 rounds
G = 8                     # chains per gang
NG = K // G               # 2 gangs
GW = G * BS               # 512 cols per gang
CCH = (L // 2) * BS       # 2048 cols per chunk in featT
NCHUNK = K
CH_DVE = 8                # chains per gang multiplied on DVE (all of them)
EMW = 16 * BS             # 1024: emit gather stream width per chunk


def crf_kernel(ctx: ExitStack, tc: tile.TileContext, outs, ins):
    nc = tc.nc
    (fwd_o, esum_o, tsum_o) = outs
    (featT_i, transT_i, stopcol_i, initcol_i, table_i, counts_i,
     oh8_i, eye_i) = ins

    const = ctx.enter_context(tc.tile_pool(name="const", bufs=1))
    efp = ctx.enter_context(tc.tile_pool(name="ef", bufs=3))
    qp = [ctx.enter_context(tc.tile_pool(name=f"q{g}", bufs=2, space="PSUM"))
          for g in range(NG)]
    zp = ctx.enter_context(tc.tile_pool(name="z", bufs=1, space="PSUM"))
    dqp = ctx.enter_context(tc.tile_pool(name="dq", bufs=1, space="PSUM"))
    smp = ctx.enter_context(tc.tile_pool(name="sm", bufs=2))

    # ---- constants ----
    mtraw = const.tile([128, T], F32)
    nc.sync.dma_start(mtraw[0:64, :], transT_i[:, :])
    nc.sync.dma_start(mtraw[64:128, :], transT_i[:, :])
    negc = const.tile([128, 1], F32)
    nc.vector.memset(negc[:, :], -CSHIFT)
    mt = const.tile([128, T], BF16)   # exp(trans.T - c), both halves, bf16
    nc.scalar.activation(mt[:, :], mtraw[:, :],
                         mybir.ActivationFunctionType.Exp, bias=negc[:, :])

    stopraw = const.tile([128, 1], F32)
    nc.sync.dma_start(stopraw[64:128, :], stopcol_i[:, :])
    stopt = const.tile([128, 1], BF16)
    nc.scalar.activation(stopt[64:128, :], stopraw[64:128, :],
                         mybir.ActivationFunctionType.Exp)

    ones_col = const.tile([128, 1], BF16)
    nc.vector.memset(ones_col[:, :], 1.0)
    initcol = const.tile([128, BS], BF16)  # e_start pattern in rows 64:128
    nc.sync.dma_start(initcol[64:128, :], initcol_i[:, :])

    table = const.tile([128, 256], F32)
    nc.sync.dma_start(table[:, :], table_i[:, :])
    counts = const.tile([128, 256], BF16)
    nc.sync.dma_start(counts[:, :], counts_i[:, :])
    eye = const.tile([T, T], BF16)
    nc.sync.dma_start(eye[:, :], eye_i[:, :])

    # gold transitions: one dot product against the replicated table
    tsc = const.tile([128, 256], F32)
    tsum = const.tile([128, 1], F32)
    nc.vector.scalar_tensor_tensor(tsc[:, :], table[:, :], 1.0, counts[:, :],
                                   op0=mybir.AluOpType.mult,
                                   op1=mybir.AluOpType.mult,
                                   accum_out=tsum[:, :])
    nc.sync.dma_start(tsum_o[:, :], tsum[:, :])

    # ---- feats (tag-major bf16, host-transposed), streamed per chunk ----
    featT = const.tile([128, (NCHUNK + 1) * CCH], F8)
    nc.vector.memset(featT[:, 0:CCH], 0.0)           # warmup chunk: feat = 0
    for c in range(1, NCHUNK + 1):
        nc.sync.dma_start(featT[:, c * CCH:(c + 1) * CCH],
                          featT_i[:, (c - 1) * CCH:c * CCH])
    featT3 = featT[:, :].rearrange("p (c x) -> p c x", x=CCH)

    oh8 = const.tile([128, NCHUNK * CCH], F8)
    for c in range(NCHUNK):
        nc.sync.dma_start(oh8[:, c * CCH:(c + 1) * CCH],
                          oh8_i[:, c * CCH:(c + 1) * CCH])

    # ---- per-gang state [128, 512]; one DVE multiply per gang per round
    #      (DVE is the only engine that can both read PSUM and do tensor x
    #      tensor; the two gangs interleave to hide each other's latency).
    sts = []
    for g in range(NG):
        st = const.tile([128, GW], BF16, tag=f"st{g}")
        nc.vector.memset(st[0:64, :], 1.0 / T)
        nc.vector.memset(st[64:128, :], 1.0 / T)
        sts.append(st)

    # log-scale stash: [h(16 chains) | t(16 chains)] x 64 batches
    zbuf = const.tile([1, 2 * K * BS], F32)

    esums = const.tile([128, NCHUNK], F32)
    nc.vector.memset(esums[:, :], 0.0)

    NBLK = R // 2
    ef_blks = [[None] * NBLK for _ in range(NG)]

    def issue_exp(g, m):
        # ef block (2 rounds): exp of the 8 chains' 1-column (64-elem) window
        eb = efp.tile([128, G * BS], BF16, tag=f"ef{g}_{m % 4}")
        r0 = 2 * m
        off = 0 if r0 < DELTA else 1
        j0 = (L - DELTA + r0) // 2 if r0 < DELTA else (r0 - DELTA) // 2
        i0 = G * g + off
        src3 = featT3[:, i0:i0 + G, j0 * BS:(j0 + 1) * BS]
        nc.scalar.activation(eb[:, :].rearrange("p (i x) -> p i x", x=BS),
                             src3, mybir.ActivationFunctionType.Exp)
        ef_blks[g][m] = eb

    dq_cur = [None, None]

    def issue_emit_part(c, part):
        # emit via one-hot diag matmuls: accumulating fp8 matmuls, then a
        # diag-extract. One accumulation group per step-parity: the PE
        # cannot alternate tile_position quadrants within a group.
        if part == 0:
            dq_e = dqp.tile([64, T], F32, tag="dqe")
            dq_o = dqp.tile([64, T], F32, tag="dqo")
            dq_cur[0], dq_cur[1] = dq_e, dq_o
        for k in range(16):
            s_in = part * 16 + k
            hs, j = s_in % 2, s_in // 2
            col0 = (c + 1) * CCH + j * BS
            ohcol0 = c * CCH + j * BS
            nc.tensor.matmul(dq_cur[hs][:, :],
                             featT[hs * 64:hs * 64 + 64, col0:col0 + BS],
                             oh8[hs * 64:hs * 64 + 64, ohcol0:ohcol0 + BS],
                             start=(s_in == hs), stop=(s_in >= 62),
                             tile_position=(hs * 64, 0))
        if part == 3:
            for hs in (0, 1):
                dsc = smp.tile([64, T], BF16, tag=f"dsc{hs}")
                nc.vector.scalar_tensor_tensor(
                    dsc[:, :], dq_cur[hs][:, :], 1.0, eye[:, :],
                    op0=mybir.AluOpType.mult, op1=mybir.AluOpType.mult,
                    accum_out=esums[hs * 64:hs * 64 + 64, c:c + 1])

    def capture(g, dst_off):
        # column-sums of gang g's 8 chains' states into zbuf
        z = zp.tile([1, GW], F32, tag=f"z{g}")
        nc.tensor.matmul(z[:, :], ones_col[64:128, :],
                         sts[g][64:128, :], tile_position=(64, 0))
        nc.vector.tensor_copy(
            zbuf[0:1, dst_off + g * GW:dst_off + (g + 1) * GW], z[:, :])

    # ---- main loop ----
    # Gang 0 only needs featT chunks 1..8 (in HBM-arrival order) so it is
    # issued LEAD rounds ahead of gang 1 and starts while gang 1's chunks
    # are still streaming in.
    LEAD = 6

    def gang_round(g, r):
        hs = r % 2
        hr = 1 - hs
        m = r // 2
        if r % 2 == 0 and m + 4 < NBLK:
            issue_exp(g, m + 4)
        if r == DELTA:
            # h-capture at position 64k (state after round DELTA-1, half 1)
            capture(g, 0)
            if g == 0:
                # chain-0 reset to the exact e_start
                nc.scalar.copy(sts[0][64:128, 0:BS], initcol[64:128, :])
        st = sts[g]
        ebv = ef_blks[g][m][hs * 64:hs * 64 + 64, :].rearrange(
            "p (i b) -> p i b", b=BS)
        q = qp[g].tile([128, GW], F32)
        nc.tensor.matmul(q[hs * 64:hs * 64 + 64, :],
                         mt[hr * 64:hr * 64 + 64, :],
                         st[hr * 64:hr * 64 + 64, :],
                         tile_position=(hr * 64, hs * 64))
        nc.vector.tensor_tensor(
            st[hs * 64:hs * 64 + 64, :].rearrange("p (i b) -> p i b", b=BS),
            q[hs * 64:hs * 64 + 64, :].rearrange("p (i b) -> p i b", b=BS),
            ebv[:, :, :],
            op=mybir.AluOpType.mult)

    for g in range(NG):
        for m in range(4):
            issue_exp(g, m)
    for r0 in range(LEAD):
        gang_round(0, r0)
    for r in range(R + LEAD):
        if r < R:
            gang_round(1, r)
        if LEAD + r < R:
            gang_round(0, LEAD + r)
        if r == 30:
            lnwarm = smp.tile([1, 1], F32, tag="lnw")
            nc.scalar.activation(lnwarm[:, :], zbuf[0:1, 0:1],
                                 mybir.ActivationFunctionType.Ln)
        if r >= 2 and (r - 2) // 4 < NCHUNK:
            issue_emit_part((r - 2) // 4, (r - 2) % 4)

    # ---- t-capture (final states live in half 1 after round R-1) ----
    for g in range(NG):
        capture(g, K * BS)
    # chain 15 terminal: stop . state overwrites its t slot (reuses z1's bank)
    tq = zp.tile([1, GW], F32, tag="z1")
    nc.tensor.matmul(tq[0:1, 0:BS], stopt[64:128, :],
                     sts[1][64:128, GW - BS:GW],
                     tile_position=(64, 0))
    nc.vector.tensor_copy(zbuf[0:1, 2 * K * BS - BS:2 * K * BS],
                          tq[0:1, 0:BS])

    # ---- fwd[b] = sum_k ln t'_k[b] - sum_{k>=1} ln h_k[b] ----
    lnz = smp.tile([1, 2 * K * BS], F32, tag="lnz")
    nc.scalar.activation(lnz[:, :], zbuf[:, :],
                         mybir.ActivationFunctionType.Ln)
    red_t = smp.tile([1, BS], F32, tag="rt")
    nc.vector.tensor_reduce(
        red_t[:, :],
        lnz[0:1, K * BS:2 * K * BS].rearrange("p (k b) -> p b k", b=BS),
        axis=mybir.AxisListType.X, op=mybir.AluOpType.add)
    red_h = smp.tile([1, BS], F32, tag="rh")
    nc.vector.tensor_reduce(
        red_h[:, :],
        lnz[0:1, BS:K * BS].rearrange("p (k b) -> p b k", b=BS),
        axis=mybir.AxisListType.X, op=mybir.AluOpType.add)
    fwd = smp.tile([1, BS], F32, tag="fwd")
    nc.vector.tensor_tensor(fwd[:, :], red_t[:, :], red_h[:, :],
                            op=mybir.AluOpType.subtract)
    nc.sync.dma_start(fwd_o[:, :], fwd[:, :])

    esum = const.tile([128, 1], F32)
    nc.vector.tensor_reduce(esum[:, :], esums[:, :],
                            axis=mybir.AxisListType.X, op=mybir.AluOpType.add)
    nc.sync.dma_start(esum_o[:, :], esum[:, :])


def build():
    nc = bacc.Bacc("TRN2", target_bir_lowering=False, debug=False)
    ins_spec = [
        ("featT", [128, NCHUNK * CCH], F8),
        ("transT", [T, T], F32),
        ("stopcol", [T, 1], F32),
        ("initcol", [T, BS], BF16),
        ("table", [128, 256], F32),
        ("counts", [128, 256], BF16),
        ("oh8", [128, NCHUNK * CCH], F8),
        ("eye", [T, T], BF16),
    ]
    outs_spec = [
        ("fwd", [1, BS], F32),
        ("esum", [128, 1], F32),
        ("tsum", [128, 1], F32),
    ]
    ins = [nc.declare_dram_parameter(n, s, d, isOutput=False).ap()
           for n, s, d in ins_spec]
    outs = [nc.declare_dram_parameter(n, s, d, isOutput=True).ap()
            for n, s, d in outs_spec]
    with tile.TileContext(nc) as tc:
        with ExitStack() as ctx:
            crf_kernel(ctx, tc, outs, ins)
    nc.compile()
    return nc


def host_prep(feats, transitions, tags, mask):
    """Build the 8 per-core input maps + host-side pad corrections."""
    assert feats.shape == (B, S, T) and transitions.shape == (T, T)
    mask_arr = np.asarray(mask)
    assert np.all(mask_arr == 1), "kernel assumes an all-ones mask"
    feats = np.asarray(feats, dtype=np.float32)
    transitions = np.asarray(transitions, dtype=np.float32)
    tags = np.asarray(tags).astype(np.int64)

    transT = np.ascontiguousarray(transitions.T)
    stopcol = np.ascontiguousarray(transitions[STOP_TAG, :].reshape(T, 1))
    initcol = np.zeros((T, BS), ml_dtypes.bfloat16)
    initcol[START_TAG, :] = 1.0
    tflat = transitions.reshape(4096)
    table = np.zeros((128, 256), np.float32)
    p_ = np.arange(128)
    table[:, :] = tflat[(p_[:, None] % 16) + 16 * np.arange(256)[None, :]]

    eye = np.eye(T, dtype=ml_dtypes.bfloat16)

    in_maps = []
    for core in range(NCORES):
        b0 = core * BS
        fb = feats[b0:b0 + BS].astype(ml_dtypes.float8_e4m3fn)  # (64,1024,64)
        tg = tags[b0:b0 + BS]

        # featT[(s%2)*64+n, c*2048 + ((s%64)//2)*64 + b] = fb[b, s, n]
        x = fb.reshape(BS, K, L // 2, 2, T)                 # (b, c, j, hs, n)
        featT = np.ascontiguousarray(
            x.transpose(3, 4, 1, 2, 0).reshape(128, NCHUNK * CCH))

        # one-hot tags in the same tag-major layout (for the emit diag-mms)
        ohsrc = np.zeros((BS, S, T), ml_dtypes.float8_e4m3fn)
        np.put_along_axis(ohsrc, tg[:, :, None], ml_dtypes.float8_e4m3fn(1.0),
                          axis=2)
        xo = ohsrc.reshape(BS, K, L // 2, 2, T)
        oh8 = np.ascontiguousarray(
            xo.transpose(3, 4, 1, 2, 0).reshape(128, NCHUNK * CCH))

        # transition-pair histogram (bf16-exact small counts)
        cur = np.concatenate([tg, np.full((BS, 1), STOP_TAG, np.int64)], 1)
        prev = np.concatenate([np.full((BS, 1), START_TAG, np.int64), tg], 1)
        lin = (cur * T + prev).reshape(-1)
        cnt = np.bincount(lin, minlength=4096)
        assert cnt.max() < 256
        counts = np.zeros((128, 256), ml_dtypes.bfloat16)
        counts[0:16, :] = cnt.reshape(256, 16).T

        in_maps.append({
            "featT": featT, "transT": transT, "stopcol": stopcol,
            "initcol": initcol, "table": table, "counts": counts,
            "oh8": oh8, "eye": eye,
        })
    return in_maps


def host_finish(results):
    fwd_total = 0.0
    gold_total = 0.0
    for r in results:
        fwd_total += float(r["fwd"].astype(np.float64).sum()) + BS * S * CSHIFT
        gold_total += float(r["esum"].astype(np.float64).sum())
        gold_total += float(r["tsum"].astype(np.float64).sum())
    return np.asarray((fwd_total - gold_total) / B, dtype=np.float32)


_NC = None


def kernel(feats, transitions, tags, mask):
    global _NC
    if _NC is None:
        _NC = build()
    in_maps = host_prep(feats, transitions, tags, mask)
    res = run_bass_kernel_spmd(_NC, in_maps, list(range(NCORES)))
    return host_finish(res.results)


if __name__ == "__main__":
    import reference
    inp = reference.setup_inputs()
    out = kernel(**{k: np.asarray(v) for k, v in inp.items()})
    print("kernel loss:", out)
